# revision 6
# baseline (speedup 1.0000x reference)
"""LIF spike kernel for Trainium2 (Bass/Tile), data-parallel over batch on 8 cores.

Reparametrized recurrence: with v_t = u_t * 2^t and host-prescaled
x'_t = x_t * 2^t (exact power-of-2 scaling), the LIF step
  u_t = tau*m_{t-1} + x_t ; s_t = u_t > 1 ; m_t = (1-s_t)*u_t   (tau = 0.5)
becomes
  v_t = m'_{t-1} + x'_t ; s_t = v_t > 2^t ; m'_t = (v_t <= 2^t) * v_t
Per (b, t) on [128, 1024] tiles:
  s   = Act sign(v - 2^t) -> i8 out tile  (off the critical chain)
  m'  = stt(v, 2^t, v, is_le, mult)       DVE
  v'  = accum-DMA: m' += x'_{t+1}         software-DGE DMA does the add
so the per-step tensor add costs no compute-engine time at all.
Host layout per core: x_core [C=128, B_loc=4, T*HW=8192] f32 (prescaled);
output i8 [C, B_loc, T*HW], spike decoded as (raw == 1).
"""

import numpy as np

import concourse.bacc as bacc
import concourse.mybir as mybir
from concourse.tile import TileContext
from concourse.bass_utils import run_bass_kernel_spmd

B, T, C, H, W = 32, 8, 128, 32, 32
HW = H * W
N_CORES = 8
B_LOC = B // N_CORES

f32 = mybir.dt.float32
i8 = mybir.dt.int8
op = mybir.AluOpType
AF = mybir.ActivationFunctionType

# spike-op engine per (b, t): 'a' = Act sign, 'v' = DVE tensor_scalar is_gt
S_ENG = [["a"] * 8 for _ in range(4)]
# reset-op style per (b, t): 'v' = stt(v,thr,v,is_le,mult) (v-only, off-chain s)
#                            's' = stt(s,1,v,is_lt,mult)   (s-coupled, cheaper)
M_STYLE = [["v"] * 7 for _ in range(4)]

_nc_cache = None


def build_nc():
    nc = bacc.Bacc("TRN2", target_bir_lowering=False)
    x = nc.dram_tensor("x", [C, B_LOC, T * HW], f32, kind="ExternalInput")
    out = nc.dram_tensor("out", [C, B_LOC, T * HW], i8, kind="ExternalOutput")

    with TileContext(nc) as tc:
        with (
            tc.tile_pool(name="vp", bufs=3) as vp,
            tc.tile_pool(name="sp_", bufs=3) as spool,
            tc.tile_pool(name="cst", bufs=1) as cst,
        ):
            # Act sign needs bias as a per-partition AP: -2^t for each t
            bias = []
            for t in range(T):
                bt = cst.tile([C, 1], f32, name=f"bias{t}")
                nc.vector.memset(bt[:], -float(2**t))
                bias.append(bt)

            # t=0 membrane: v_0 = x'_0
            v_cur = []
            for b in range(B_LOC):
                vt = vp.tile([C, HW], f32, tag=f"v{b}")
                nc.sync.dma_start(out=vt[:], in_=x[:, b, 0:HW])
                v_cur.append(vt)

            for t in range(T):
                thr = float(2**t)
                for b in range(B_LOC):
                    v = v_cur[b]
                    # spike output: s = v > 2^t, as i8, spike == 1
                    st = spool.tile([C, HW], i8, tag=f"s{b}")
                    if S_ENG[b][t] == "a":
                        nc.scalar.activation(
                            st[:], v[:], AF.Sign, bias=bias[t][:], scale=1.0
                        )
                    else:
                        nc.vector.tensor_scalar(st[:], v[:], thr, None, op.is_gt)
                    nc.sync.dma_start(
                        out=out[:, b, t * HW : (t + 1) * HW], in_=st[:]
                    )
                    if t == T - 1:
                        continue
                    # m' = (v <= 2^t) * v, then v' = m' + x'_{t+1} via accum-DMA
                    mt = vp.tile([C, HW], f32, tag=f"v{b}")
                    if M_STYLE[b][t] == "v":
                        nc.vector.scalar_tensor_tensor(
                            mt[:], v[:], thr, v[:], op.is_le, op.mult
                        )
                    else:
                        nc.vector.scalar_tensor_tensor(
                            mt[:], st[:], 1.0, v[:], op.is_lt, op.mult
                        )
                    nc.gpsimd.dma_start(
                        out=mt[:],
                        in_=x[:, b, (t + 1) * HW : (t + 2) * HW],
                        accum_op=op.add,
                    )
                    v_cur[b] = mt
    nc.compile()
    return nc


def make_in_maps(x: np.ndarray) -> list[dict]:
    xs = np.ascontiguousarray(x).reshape(B, T, C, HW)
    # prescale x'_t = x_t * 2^t (exact in f32)
    scale = (2.0 ** np.arange(T, dtype=np.float32)).astype(np.float32)
    xs = (xs * scale[None, :, None, None]).astype(np.float32)
    return [
        {
            "x": np.ascontiguousarray(
                xs[i * B_LOC : (i + 1) * B_LOC].transpose(2, 0, 1, 3)
            ).reshape(C, B_LOC, T * HW)
        }
        for i in range(N_CORES)
    ]


def kernel(x: np.ndarray) -> np.ndarray:
    global _nc_cache
    if _nc_cache is None:
        _nc_cache = build_nc()
    res = run_bass_kernel_spmd(_nc_cache, make_in_maps(x), list(range(N_CORES)))
    # out[c, b_loc, t*HW+hw] -> [b, t, c, hw]; spike iff raw == 1
    parts = [
        (res.results[i]["out"].reshape(C, B_LOC, T, HW) == 1).transpose(1, 2, 0, 3)
        for i in range(N_CORES)
    ]
    full = np.concatenate(parts, axis=0)
    return full.reshape(B, T, C, H, W).astype(np.float32)
